# revision 5
# baseline (speedup 1.0000x reference)
"""Trainium2 Bass kernel for nn_Attention_38491496907192.

LayerNorm -> QKV projection -> cosine-sim causal attention (8 heads) -> out
projection, for x [2, 2048, 1024], w_qkv [1024, 1536], w_out [512, 1024].

Sharding (8 NeuronCores): core i handles batch i//4 and head pair
(i%4)*2 .. +2  (data parallel over batch, tensor parallel over heads:
w_qkv split column-wise by head, w_out row-wise). Each core emits a
partial [2048, 1024] output (its heads' contribution through the row
slice of w_out); the host sums the 4 partials per batch.

Per-core pipeline (all matmul operands fp16, fp32 PSUM accumulation):
  1. LayerNorm stats via bn_stats in [n, dim] layout, fused
     (x-mu)*rsig normalize to fp16, DMA-transpose to xn^T strips.
     ln_w is folded into the weights host-side; ln_b becomes per-output
     biases (exact since LN output is affine in the weights).
  2. qT/kT = (xn @ wq/k)^T computed directly in [feat, n] layout;
     v in [n, feat] layout with interleaved ones-columns
     [v_h0 | 1 | v_h1 | 1] so P@V also yields softmax denominators.
  3. L2 norms of q/k per head via ones-vector matmuls (partition-dim
     reduction on PE), rsqrt, partition-broadcast, fp16 scale.
  4. Attention per (q-megablock m, head h) in S^T layout [k, q]:
     scores = k^T.T @ q^T (no transposes anywhere), exp(8*s) on ACT,
     causal diagonal blocks masked by precomputed 0/1 fp16 masks on DVE,
     P@V accumulation -> O^T [65, 512] (row 64 = denominator), divide,
     out-projection row-slice, fp16 partial out.

Cosine-sim attention is bounded (|s| <= 8) so softmax needs no row-max:
exp values lie in [e^-8, e^8], fp32 sums are exact enough, which is what
makes the transpose-free S^T layout possible.
"""

import sys

sys.path.insert(0, "/opt/trn_rl_repo")

import numpy as np
from contextlib import ExitStack

import concourse.bass as bass
import concourse.tile as tile
from concourse import bacc, mybir
from concourse import bass_utils

AF = mybir.ActivationFunctionType
OP = mybir.AluOpType
F16 = mybir.dt.float16
F32 = mybir.dt.float32

B, N, DIM = 2, 2048, 1024
H, D = 8, 64
P = 128
NBLK = N // P            # 16 row blocks
KT = DIM // P            # 8 contraction tiles
QM = N // 512            # 4 query megablocks
N_CORES = 8
LN_EPS = 1e-5
L2_EPS = 1e-12
SCALE = 8.0

_CACHE = {}


def build_nc():
    nc = bacc.Bacc("TRN2", target_bir_lowering=False, debug=False,
                   num_devices=N_CORES)
    x_d = nc.dram_tensor("x", [N, DIM], F32, kind="ExternalInput").ap()
    wqkv_d = nc.dram_tensor("wqkv", [P, KT, 384], F16, kind="ExternalInput").ap()
    wout_d = nc.dram_tensor("wout", [P, DIM], F16, kind="ExternalInput").ap()
    qkb_d = nc.dram_tensor("qkb", [P, 2], F32, kind="ExternalInput").ap()
    vb_d = nc.dram_tensor("vb", [1, P], F32, kind="ExternalInput").ap()
    y_d = nc.dram_tensor("y", [N, DIM], F16, kind="ExternalOutput").ap()

    with tile.TileContext(nc) as tc, ExitStack() as ctx:
        const = ctx.enter_context(tc.tile_pool(name="const", bufs=1))
        wq_sb = const.tile([P, KT, 384], F16, name="wq")
        wout_sb = const.tile([P, DIM], F16, name="wout")
        qkb_sb = const.tile([P, 2], F32, name="qkb")
        vb1 = const.tile([1, P], F32, name="vb1")
        vb_sb = const.tile([P, P], F32, name="vb")
        masks = const.tile([P, 4, 512], F16, name="masks")
        ones_c = const.tile([P, 1], F16, name="ones")
        zero_c = const.tile([P, 1], F32, name="zero")
        eps_ln = const.tile([P, 1], F32, name="epsln")
        eps_l2 = const.tile([1, 1], F32, name="epsl2")
        xnT = [const.tile([P, N], F16, name=f"xnT{kt}") for kt in range(KT)]
        qT = const.tile([P, N], F16, name="qT")
        kT = const.tile([P, N], F16, name="kT")
        V = const.tile([P, NBLK, 2 * (D + 1)], F16, name="V")

        nc.sync.dma_start(wq_sb[:], wqkv_d[:])
        nc.sync.dma_start(wout_sb[:], wout_d[:])
        nc.sync.dma_start(qkb_sb[:], qkb_d[:])
        nc.sync.dma_start(vb1[:], vb_d[:])
        nc.gpsimd.partition_broadcast(vb_sb[:], vb1[:])
        nc.vector.memset(ones_c[:], 1.0)
        nc.vector.memset(zero_c[:], 0.0)
        nc.vector.memset(eps_ln[:], LN_EPS)
        nc.vector.memset(eps_l2[:], L2_EPS)
        for j in range(4):
            # keep where q - k >= 0 i.e. (y - x - 128*j) >= 0, else 0
            nc.gpsimd.memset(masks[:, j, :], 1.0)
            nc.gpsimd.affine_select(
                out=masks[:, j, :], in_=masks[:, j, :],
                compare_op=OP.is_ge, fill=0.0, base=-128 * j,
                pattern=[[1, 512]], channel_multiplier=-1)

        # ---- Phase 1: LayerNorm + transpose ----
        xpool = ctx.enter_context(tc.tile_pool(name="xp", bufs=3))
        statp = ctx.enter_context(tc.tile_pool(name="stat", bufs=6))
        xnpool = ctx.enter_context(tc.tile_pool(name="xn", bufs=3))
        for nb in range(NBLK):
            xt = xpool.tile([P, DIM], F32, name="xt")
            nc.sync.dma_start(xt[:], x_d[nb * P:(nb + 1) * P, :])
            st = statp.tile([P, 2, 6], F32, name="st")
            nc.vector.bn_stats(st[:, 0, :], xt[:, 0:512])
            nc.vector.bn_stats(st[:, 1, :], xt[:, 512:1024])
            mv = statp.tile([P, 2], F32, name="mv")
            nc.vector.bn_aggr(mv[:], st[:])
            sd = statp.tile([P, 1], F32, name="sd")
            nc.scalar.activation(sd[:], mv[:, 1:2], AF.Sqrt, bias=eps_ln[:])
            rs = statp.tile([P, 1], F32, name="rs")
            nc.vector.reciprocal(rs[:], sd[:])
            xnt = xnpool.tile([P, DIM], F16, name="xnt")
            nc.vector.tensor_scalar(out=xnt[:], in0=xt[:],
                                    scalar1=mv[:, 0:1], scalar2=rs[:],
                                    op0=OP.subtract, op1=OP.mult)
            for kt in range(KT):
                nc.sync.dma_start(xnT[kt][:, nb * P:(nb + 1) * P],
                                  xnt[:, kt * P:(kt + 1) * P], transpose=True)

        # ---- Phase 2: V projection ----
        with tc.tile_pool(name="psV", bufs=2, space="PSUM") as psV:
            for nb in range(NBLK):
                pv = psV.tile([P, P], F32, name="pv")
                for kt in range(KT):
                    nc.tensor.matmul(pv[:],
                                     lhsT=xnT[kt][:, nb * P:(nb + 1) * P],
                                     rhs=wq_sb[:, kt, 256:384],
                                     start=(kt == 0), stop=(kt == KT - 1))
                nc.vector.tensor_tensor(V[:, nb, 0:D], pv[:, 0:D],
                                        vb_sb[:, 0:D], OP.add)
                nc.vector.tensor_tensor(V[:, nb, D + 1:2 * D + 1],
                                        pv[:, D:2 * D],
                                        vb_sb[:, D:2 * D], OP.add)
        nc.vector.memset(V[:, :, D:D + 1], 1.0)
        nc.vector.memset(V[:, :, 2 * D + 1:2 * D + 2], 1.0)

        # ---- Phase 3: qT / kT projections ----
        psA = ctx.enter_context(tc.tile_pool(name="psA", bufs=4, space="PSUM"))
        for j in range(QM):
            js = slice(j * 512, (j + 1) * 512)
            pq = psA.tile([P, 512], F32, name="pa")
            for kt in range(KT):
                nc.tensor.matmul(pq[:], lhsT=wq_sb[:, kt, 0:128],
                                 rhs=xnT[kt][:, js],
                                 start=(kt == 0), stop=(kt == KT - 1))
            nc.vector.tensor_scalar(out=qT[:, js], in0=pq[:],
                                    scalar1=qkb_sb[:, 0:1], scalar2=None,
                                    op0=OP.add)
            pk = psA.tile([P, 512], F32, name="pa")
            for kt in range(KT):
                nc.tensor.matmul(pk[:], lhsT=wq_sb[:, kt, 128:256],
                                 rhs=xnT[kt][:, js],
                                 start=(kt == 0), stop=(kt == KT - 1))
            nc.vector.tensor_scalar(out=kT[:, js], in0=pk[:],
                                    scalar1=qkb_sb[:, 1:2], scalar2=None,
                                    op0=OP.add)

        # ---- Phase 4: l2-normalize q, k per head ----
        sqp = ctx.enter_context(tc.tile_pool(name="sq", bufs=3))
        rbp = ctx.enter_context(tc.tile_pool(name="rb", bufs=3))
        smallp = ctx.enter_context(tc.tile_pool(name="sm", bufs=8))
        with tc.tile_pool(name="psN", bufs=2, space="PSUM") as psN:
            for T in (qT, kT):
                for j in range(QM):
                    js = slice(j * 512, (j + 1) * 512)
                    sq = sqp.tile([P, 512], F16, name="sq")
                    nc.vector.tensor_tensor(sq[:], T[:, js], T[:, js], OP.mult)
                    rbt = rbp.tile([P, 512], F16, name="rbt")
                    for h in range(2):
                        hs = slice(h * 64, (h + 1) * 64)
                        pn = psN.tile([1, 512], F32, name="pn")
                        nc.tensor.matmul(pn[:], lhsT=ones_c[hs, :],
                                         rhs=sq[hs, :], start=True, stop=True)
                        sqv = smallp.tile([1, 512], F32, name="sqv")
                        nc.scalar.activation(sqv[:], pn[:], AF.Sqrt, bias=eps_l2[:])
                        rn = smallp.tile([1, 512], F16, name="rn")
                        with nc.allow_low_precision(
                                reason="1/||q|| is O(0.1), fp16 is plenty"):
                            nc.vector.reciprocal(rn[:], sqv[:])
                        # partition_broadcast requires a partition-0 source and
                        # a partition-0 destination base (HW reads the wrong
                        # partition otherwise) -> h1 goes via a temp + DMA.
                        if h == 0:
                            nc.gpsimd.partition_broadcast(rbt[0:64, :], rn[:])
                        else:
                            rtmp = rbp.tile([64, 512], F16, name="rtmp")
                            nc.gpsimd.partition_broadcast(rtmp[:], rn[:])
                            nc.sync.dma_start(rbt[64:128, :], rtmp[:])
                    nc.vector.tensor_tensor(T[:, js], T[:, js], rbt[:], OP.mult)

        # ---- Phase 5: attention + out-projection ----
        epool = ctx.enter_context(tc.tile_pool(name="ep", bufs=20))
        psB = ctx.enter_context(tc.tile_pool(name="psB", bufs=2, space="PSUM"))
        ofp = ctx.enter_context(tc.tile_pool(name="of", bufs=3))
        o1p = ctx.enter_context(tc.tile_pool(name="o1", bufs=2))
        ocp = ctx.enter_context(tc.tile_pool(name="oc", bufs=3))
        rlp = ctx.enter_context(tc.tile_pool(name="rl", bufs=3))
        outp = ctx.enter_context(tc.tile_pool(name="out", bufs=3))

        for m in range(QM):
            KB = 4 * m + 4
            ms = slice(m * 512, (m + 1) * 512)
            ocat = ocp.tile([P, 512], F16, name="ocat")
            for h in range(2):
                hs = slice(h * 64, (h + 1) * 64)
                Es = []
                for kb in range(KB):
                    ps = psA.tile([P, 512], F32, name="pa")
                    nc.tensor.matmul(ps[:],
                                     lhsT=kT[hs, kb * P:(kb + 1) * P],
                                     rhs=qT[hs, ms], start=True, stop=True)
                    E = epool.tile([P, 512], F16, name="E")
                    nc.scalar.activation(E[:], ps[:], AF.Exp, bias=zero_c[:], scale=SCALE)
                    if kb >= 4 * m:
                        nc.vector.tensor_tensor(E[:], E[:],
                                                masks[:, kb - 4 * m, :], OP.mult)
                    Es.append(E)
                po = psB.tile([D + 1, 512], F32, name="po")
                for kb in range(KB):
                    nc.tensor.matmul(po[:],
                                     lhsT=V[:, kb, h * (D + 1):(h + 1) * (D + 1)],
                                     rhs=Es[kb][:],
                                     start=(kb == 0), stop=(kb == KB - 1))
                of = ofp.tile([D + 1, 512], F32, name="of")
                nc.vector.tensor_copy(of[:], po[:])
                l0 = smallp.tile([1, 512], F32, name="l0")
                nc.sync.dma_start(l0[0:1, :], of[D:D + 1, :])
                rb65 = rlp.tile([D, 512], F32, name="rb65")
                nc.gpsimd.partition_broadcast(rb65[:], l0[:])
                nc.vector.reciprocal(rb65[:], rb65[:])
                if h == 0:
                    nc.vector.tensor_tensor(ocat[0:D, :], of[0:D, :],
                                            rb65[:], OP.mult)
                else:
                    ot1 = o1p.tile([D, 512], F16, name="ot1")
                    nc.vector.tensor_tensor(ot1[:], of[0:D, :], rb65[:], OP.mult)
                    nc.sync.dma_start(ocat[D:2 * D, :], ot1[:])
            for qb in range(4):
                row = (m * 4 + qb) * P
                yt = outp.tile([P, DIM], F16, name="yt")
                for ns in range(2):
                    py = psA.tile([P, 512], F32, name="pa")
                    nc.tensor.matmul(py[:],
                                     lhsT=ocat[:, qb * P:(qb + 1) * P],
                                     rhs=wout_sb[:, ns * 512:(ns + 1) * 512],
                                     start=True, stop=True)
                    nc.vector.tensor_copy(yt[:, ns * 512:(ns + 1) * 512], py[:])
                nc.sync.dma_start(y_d[row:row + P, :], yt[:])

    nc.compile()
    return nc


def make_in_maps(x, ln_w, ln_b, w_qkv, w_out):
    x = np.asarray(x, np.float32)
    ln_w = np.asarray(ln_w, np.float32)
    ln_b = np.asarray(ln_b, np.float32)
    w_qkv = np.asarray(w_qkv, np.float32)
    w_out = np.asarray(w_out, np.float32)
    in_maps = []
    for core in range(N_CORES):
        b, h0 = core // 4, (core % 4) * 2
        cs = [slice(base + h0 * D, base + (h0 + 2) * D)
              for base in (0, 512, 1024)]
        w_parts = [w_qkv[:, c] * ln_w[:, None] for c in cs]
        wcat = np.concatenate(w_parts, axis=1).astype(np.float16)
        wcat = np.ascontiguousarray(
            wcat.reshape(KT, P, 384).transpose(1, 0, 2))
        qb_ = (ln_b @ w_qkv[:, cs[0]]).astype(np.float32)
        kb_ = (ln_b @ w_qkv[:, cs[1]]).astype(np.float32)
        vb_ = (ln_b @ w_qkv[:, cs[2]]).astype(np.float32)
        in_maps.append({
            "x": np.ascontiguousarray(x[b]),
            "wqkv": wcat,
            "wout": np.ascontiguousarray(
                w_out[h0 * D:(h0 + 2) * D]).astype(np.float16),
            "qkb": np.ascontiguousarray(np.stack([qb_, kb_], axis=1)),
            "vb": np.ascontiguousarray(vb_[None, :]),
        })
    return in_maps


def kernel(x, ln_w, ln_b, w_qkv, w_out):
    if "nc" not in _CACHE:
        _CACHE["nc"] = build_nc()
    nc = _CACHE["nc"]
    in_maps = make_in_maps(x, ln_w, ln_b, w_qkv, w_out)
    res = bass_utils.run_bass_kernel_spmd(nc, in_maps,
                                          core_ids=list(range(N_CORES)))
    y = np.zeros((B, N, DIM), np.float32)
    for core in range(N_CORES):
        y[core // 4] += res.results[core]["y"].astype(np.float32)
    return y


# revision 15
# speedup vs baseline: 1.2931x; 1.2931x over previous
"""Trainium2 Bass kernel for nn_Attention_38491496907192.

LayerNorm -> QKV projection -> cosine-sim causal attention (8 heads) -> out
projection, for x [2, 2048, 1024], w_qkv [1024, 1536], w_out [512, 1024].

Sharding (8 NeuronCores): core i handles batch i//4 and head pair
(i%4)*2 .. +2  (data parallel over batch, tensor parallel over heads:
w_qkv split column-wise by head, w_out row-wise). Each core emits a
partial [2048, 1024] output (its heads' contribution through the row
slice of w_out); the host sums the 4 partials per batch.

Per-core pipeline (all matmul operands fp16, fp32 PSUM accumulation):
  1. LayerNorm stats via bn_stats in [n, dim] layout, fused
     (x-mu)*rsig normalize to fp16, DMA-transpose to xn^T strips.
     ln_w is folded into the weights host-side; ln_b becomes per-output
     biases (exact since LN output is affine in the weights).
  2. qT/kT = (xn @ wq/k)^T computed directly in [feat, n] layout;
     v in [n, feat] layout with interleaved ones-columns
     [v_h0 | 1 | v_h1 | 1] so P@V also yields softmax denominators.
  3. L2 norms of q/k per head via ones-vector matmuls (partition-dim
     reduction on PE), rsqrt, partition-broadcast, fp16 scale.
  4. Attention per (q-megablock m, head h) in S^T layout [k, q]:
     scores = k^T.T @ q^T (no transposes anywhere), exp(8*s) on ACT,
     causal diagonal blocks masked by precomputed 0/1 fp16 masks on DVE,
     P@V accumulation -> O^T [65, 512] (row 64 = denominator), divide,
     out-projection row-slice, fp16 partial out.

Cosine-sim attention is bounded (|s| <= 8) so softmax needs no row-max:
exp values lie in [e^-8, e^8], fp32 sums are exact enough, which is what
makes the transpose-free S^T layout possible.
"""

import sys

sys.path.insert(0, "/opt/trn_rl_repo")

import numpy as np
from contextlib import ExitStack

import concourse.bass as bass
import concourse.tile as tile
from concourse import bacc, mybir
from concourse import bass_utils

AF = mybir.ActivationFunctionType
OP = mybir.AluOpType
F16 = mybir.dt.float16
F32 = mybir.dt.float32

B, N, DIM = 2, 2048, 1024
H, D = 8, 64
P = 128
NBLK = N // P            # 16 row blocks
KT = DIM // P            # 8 contraction tiles
QM = N // 512            # 4 query megablocks
N_CORES = 8
LN_EPS = 1e-5
L2_EPS = 1e-12
SCALE = 8.0

_CACHE = {}


def build_nc(stop_after=99):
    nc = bacc.Bacc("TRN2", target_bir_lowering=False, debug=False,
                   num_devices=N_CORES)
    xT_d = nc.dram_tensor("xT", [DIM, N], F16, kind="ExternalInput").ap()
    wqkv_d = nc.dram_tensor("wqkv", [P, KT, 384], F16, kind="ExternalInput").ap()
    wout_d = nc.dram_tensor("wout", [P, DIM], F16, kind="ExternalInput").ap()
    qkb_d = nc.dram_tensor("qkb", [P, 2], F32, kind="ExternalInput").ap()
    vb_d = nc.dram_tensor("vb", [1, P], F32, kind="ExternalInput").ap()
    y_d = nc.dram_tensor("y", [N, DIM], F16, kind="ExternalOutput").ap()

    with tile.TileContext(nc) as tc, ExitStack() as ctx:
        const = ctx.enter_context(tc.tile_pool(name="const", bufs=1))
        wq_sb = const.tile([P, KT, 384], F16, name="wq")
        wout_sb = const.tile([P, DIM], F16, name="wout")
        qkb_sb = const.tile([P, 2], F32, name="qkb")
        vb1 = const.tile([1, P], F32, name="vb1")
        vb_sb = const.tile([P, P], F32, name="vb")
        ones_c = const.tile([P, 1], F16, name="ones")
        zero_c = const.tile([P, 1], F32, name="zero")
        eps_ln = const.tile([P, 1], F32, name="epsln")
        eps_l2 = const.tile([1, 1], F32, name="epsl2")
        eps_ln1 = const.tile([1, 1], F32, name="epsln1")
        xnT = [const.tile([P, N], F16, name=f"xnT{kt}") for kt in range(KT)]
        qT = const.tile([P, N], F16, name="qT")
        kT = const.tile([P, N], F16, name="kT")
        V = const.tile([P, NBLK, 2 * (D + 1)], F16, name="V")

        nc.sync.dma_start(wq_sb[:], wqkv_d[:])
        nc.sync.dma_start(wout_sb[:], wout_d[:])
        nc.sync.dma_start(qkb_sb[:], qkb_d[:])
        nc.sync.dma_start(vb1[:], vb_d[:])
        nc.gpsimd.partition_broadcast(vb_sb[:], vb1[:])
        nc.vector.memset(ones_c[:], 1.0)
        nc.vector.memset(zero_c[:], 0.0)
        nc.vector.memset(eps_ln[:], LN_EPS)
        nc.vector.memset(eps_l2[:], L2_EPS)
        nc.vector.memset(eps_ln1[:], LN_EPS)

        # ---- Phase 1: LayerNorm in transposed layout ----
        # x^T arrives pre-transposed from the host (layout prep). LN stats
        # (sums over dim) are partition-dim reductions here, done on PE with
        # a ones vector; sum(x^2) via a DVE square + same trick. Then
        # xn^T = x^T * a + b with a = rsig, b = -mu*rsig broadcast across
        # partitions, written straight into the fp16 xn^T strips.
        statp = ctx.enter_context(tc.tile_pool(name="stat", bufs=1))
        a_b = const.tile([P, N], F16, name="a_b")
        b_b = const.tile([P, N], F16, name="b_b")
        with tc.tile_pool(name="xTp", bufs=KT, space="SBUF") as xTp, \
             tc.tile_pool(name="xsq", bufs=2) as xsqp, \
             tc.tile_pool(name="psS", bufs=1, space="PSUM") as psS:
            xTs = []
            for kt in range(KT):
                xts = xTp.tile([P, N], F16, name="xts")
                nc.sync.dma_start(xts[:], xT_d[kt * P:(kt + 1) * P, :])
                xTs.append(xts)
            # one PSUM bank per (j, stat): accumulate over all kt on PE
            pstat = [[psS.tile([1, 512], F32, name=f"ps_{j}_{s}")
                      for s in range(2)] for j in range(QM)]
            for kt in range(KT):
                sq = xsqp.tile([P, N], F16, name="sq")
                nc.vector.tensor_tensor(sq[:], xTs[kt][:], xTs[kt][:], OP.mult)
                for j in range(QM):
                    js = slice(j * 512, (j + 1) * 512)
                    nc.tensor.matmul(pstat[j][0][:], lhsT=ones_c[:],
                                     rhs=xTs[kt][:, js],
                                     start=(kt == 0), stop=(kt == KT - 1))
                    nc.tensor.matmul(pstat[j][1][:], lhsT=ones_c[:],
                                     rhs=sq[:, js],
                                     start=(kt == 0), stop=(kt == KT - 1))
            # mu = s1/DIM ; var = s2/DIM - mu^2 ; a = rsqrt(var+eps) ; b = -mu*a
            t1 = statp.tile([1, N], F32, name="t1")   # mu
            t2 = statp.tile([1, N], F32, name="t2")   # var -> av
            t3 = statp.tile([1, N], F32, name="t3")   # mu^2 -> sdv -> bv
            for j in range(QM):
                js = slice(j * 512, (j + 1) * 512)
                nc.vector.tensor_scalar(out=t1[:, js], in0=pstat[j][0][:],
                                        scalar1=1.0 / DIM, scalar2=None,
                                        op0=OP.mult)
                nc.vector.tensor_scalar(out=t2[:, js], in0=pstat[j][1][:],
                                        scalar1=1.0 / DIM, scalar2=None,
                                        op0=OP.mult)
            nc.vector.tensor_tensor(t3[:], t1[:], t1[:], OP.mult)
            nc.vector.tensor_tensor(t2[:], t2[:], t3[:], OP.subtract)
            nc.scalar.activation(t3[:], t2[:], AF.Sqrt, bias=eps_ln1[:])
            nc.vector.reciprocal(t2[:], t3[:])
            nc.vector.tensor_tensor(t3[:], t1[:], t2[:], OP.mult)
            av16 = statp.tile([1, N], F16, name="av16")
            nc.vector.tensor_copy(av16[:], t2[:])
            bv16 = statp.tile([1, N], F16, name="bv16")
            nc.vector.tensor_scalar(out=bv16[:], in0=t3[:], scalar1=-1.0,
                                    scalar2=None, op0=OP.mult)
            nc.gpsimd.partition_broadcast(a_b[:], av16[:])
            nc.gpsimd.partition_broadcast(b_b[:], bv16[:])
            for kt in range(KT):
                tmp = xsqp.tile([P, N], F16, name="sq")
                nc.vector.tensor_tensor(tmp[:], xTs[kt][:], a_b[:], OP.mult)
                nc.vector.tensor_tensor(xnT[kt][:], tmp[:], b_b[:], OP.add)

        # ---- Phases 2-5: per-j wavefront ----
        # Everything after LN is emitted per query-megablock j so attention
        # on early columns overlaps projection/normalization of later ones.
        psA = ctx.enter_context(tc.tile_pool(name="psA", bufs=2, space="PSUM"))
        psC = ctx.enter_context(tc.tile_pool(name="psC", bufs=2, space="PSUM"))
        psB = ctx.enter_context(tc.tile_pool(name="psB", bufs=1, space="PSUM"))
        psN = ctx.enter_context(tc.tile_pool(name="psN", bufs=1, space="PSUM"))
        sqp = ctx.enter_context(tc.tile_pool(name="sq", bufs=2))
        rbp = ctx.enter_context(tc.tile_pool(name="rb", bufs=2))
        smallp = ctx.enter_context(tc.tile_pool(name="sm", bufs=6))
        epool = ctx.enter_context(tc.tile_pool(name="ep", bufs=10))
        ofp = ctx.enter_context(tc.tile_pool(name="of", bufs=2))
        o1p = ctx.enter_context(tc.tile_pool(name="o1", bufs=2))
        ocp = ctx.enter_context(tc.tile_pool(name="oc", bufs=3))
        rlp = ctx.enter_context(tc.tile_pool(name="rl", bufs=2))
        outp = ctx.enter_context(tc.tile_pool(name="out", bufs=3))

        nc.vector.memset(V[:, :, D:D + 1], 1.0)
        nc.vector.memset(V[:, :, 2 * D + 1:2 * D + 2], 1.0)

        for j in range(QM):
            js = slice(j * 512, (j + 1) * 512)
            # -- qT / kT projection for columns js --
            for ti, T in enumerate((qT, kT)):
                c0 = ti * 128
                pq = psA.tile([P, 512], F32, name="pa")
                for kt in range(KT):
                    nc.tensor.matmul(pq[:], lhsT=wq_sb[:, kt, c0:c0 + 128],
                                     rhs=xnT[kt][:, js],
                                     start=(kt == 0), stop=(kt == KT - 1))
                nc.vector.tensor_scalar(out=T[:, js], in0=pq[:],
                                        scalar1=qkb_sb[:, ti:ti + 1],
                                        scalar2=None, op0=OP.add)
            # -- V projection for row blocks of this j --
            for nb in range(4 * j, 4 * j + 4):
                pv = psA.tile([P, 512], F32, name="pa")
                for kt in range(KT):
                    nc.tensor.matmul(pv[:, 0:P],
                                     lhsT=xnT[kt][:, nb * P:(nb + 1) * P],
                                     rhs=wq_sb[:, kt, 256:384],
                                     start=(kt == 0), stop=(kt == KT - 1))
                nc.vector.tensor_tensor(V[:, nb, 0:D], pv[:, 0:D],
                                        vb_sb[:, 0:D], OP.add)
                nc.vector.tensor_tensor(V[:, nb, D + 1:2 * D + 1],
                                        pv[:, D:2 * D],
                                        vb_sb[:, D:2 * D], OP.add)
            # -- l2 norms of q/k columns js --
            for T in (qT, kT):
                sq = sqp.tile([P, 512], F16, name="sq")
                nc.vector.tensor_tensor(sq[:], T[:, js], T[:, js], OP.mult)
                rbt = rbp.tile([P, 512], F16, name="rbt")
                for h in range(2):
                    hs = slice(h * 64, (h + 1) * 64)
                    pn = psN.tile([1, 512], F32, name="pn")
                    nc.tensor.matmul(pn[:], lhsT=ones_c[hs, :],
                                     rhs=sq[hs, :], start=True, stop=True)
                    sqv = smallp.tile([1, 512], F32, name="sqv")
                    nc.scalar.activation(sqv[:], pn[:], AF.Sqrt, bias=eps_l2[:])
                    rn = smallp.tile([1, 512], F16, name="rn")
                    with nc.allow_low_precision(
                            reason="1/||q|| is O(0.1), fp16 is plenty"):
                        nc.vector.reciprocal(rn[:], sqv[:])
                    # partition_broadcast needs partition-0 src and dst base
                    if h == 0:
                        nc.gpsimd.partition_broadcast(rbt[0:64, :], rn[:])
                    else:
                        rtmp = rbp.tile([64, 512], F16, name="rtmp")
                        nc.gpsimd.partition_broadcast(rtmp[:], rn[:])
                        nc.sync.dma_start(rbt[64:128, :], rtmp[:])
                nc.vector.tensor_tensor(T[:, js], T[:, js], rbt[:], OP.mult)
            # -- attention for query megablock m = j --
            KB = 4 * j + 4
            ocat = ocp.tile([P, 512], F16, name="ocat")
            of2 = ofp.tile([D + 1, 1024], F32, name="of")
            for h in range(2):
                hs = slice(h * 64, (h + 1) * 64)
                Es = []
                for kp in range(KB // 2):
                    ps2 = psC.tile([P, 1024], F32, name="pc")
                    for half in range(2):
                        kb = 2 * kp + half
                        nc.tensor.matmul(ps2[:, half * 512:(half + 1) * 512],
                                         lhsT=kT[hs, kb * P:(kb + 1) * P],
                                         rhs=qT[hs, js], start=True, stop=True)
                    E2 = epool.tile([P, 1024], F16, name="E")
                    nc.scalar.activation(E2[:], ps2[:], AF.Exp,
                                         bias=zero_c[:], scale=SCALE)
                    for half in range(2):
                        kb = 2 * kp + half
                        if kb >= 4 * j:
                            nc.gpsimd.affine_select(
                                out=E2[:, half * 512:(half + 1) * 512],
                                in_=E2[:, half * 512:(half + 1) * 512],
                                compare_op=OP.is_ge, fill=0.0,
                                base=-128 * (kb - 4 * j),
                                pattern=[[1, 512]], channel_multiplier=-1)
                    Es.append(E2)
                po = psB.tile([D + 1, 512], F32, name="po")
                for kp in range(KB // 2):
                    for half in range(2):
                        kb = 2 * kp + half
                        nc.tensor.matmul(
                            po[:],
                            lhsT=V[:, kb, h * (D + 1):(h + 1) * (D + 1)],
                            rhs=Es[kp][:, half * 512:(half + 1) * 512],
                            start=(kb == 0), stop=(kb == KB - 1))
                nc.vector.tensor_copy(of2[:, h * 512:(h + 1) * 512], po[:])
            l0 = smallp.tile([1, 1024], F32, name="l0")
            nc.sync.dma_start(l0[0:1, :], of2[D:D + 1, :])
            nc.vector.reciprocal(l0[:], l0[:])
            rb65 = rlp.tile([D, 1024], F32, name="rb65")
            nc.gpsimd.partition_broadcast(rb65[:], l0[:])
            nc.vector.tensor_tensor(ocat[0:D, :], of2[0:D, 0:512],
                                    rb65[:, 0:512], OP.mult)
            ot1 = o1p.tile([D, 512], F16, name="ot1")
            nc.vector.tensor_tensor(ot1[:], of2[0:D, 512:1024],
                                    rb65[:, 512:1024], OP.mult)
            nc.sync.dma_start(ocat[D:2 * D, :], ot1[:])
            # -- out-projection for this megablock --
            for qb in range(4):
                row = (j * 4 + qb) * P
                yt = outp.tile([P, DIM], F16, name="yt")
                for ns in range(2):
                    py = psA.tile([P, 512], F32, name="pa")
                    nc.tensor.matmul(py[:],
                                     lhsT=ocat[:, qb * P:(qb + 1) * P],
                                     rhs=wout_sb[:, ns * 512:(ns + 1) * 512],
                                     start=True, stop=True)
                    if ns == 0:
                        nc.vector.tensor_copy(yt[:, 0:512], py[:])
                    else:
                        nc.scalar.copy(yt[:, 512:1024], py[:])
                nc.sync.dma_start(y_d[row:row + P, :], yt[:])

    nc.compile()
    return nc


def make_in_maps(x, ln_w, ln_b, w_qkv, w_out):
    x = np.asarray(x, np.float32)
    ln_w = np.asarray(ln_w, np.float32)
    ln_b = np.asarray(ln_b, np.float32)
    w_qkv = np.asarray(w_qkv, np.float32)
    w_out = np.asarray(w_out, np.float32)
    in_maps = []
    for core in range(N_CORES):
        b, h0 = core // 4, (core % 4) * 2
        cs = [slice(base + h0 * D, base + (h0 + 2) * D)
              for base in (0, 512, 1024)]
        w_parts = [w_qkv[:, c] * ln_w[:, None] for c in cs]
        wcat = np.concatenate(w_parts, axis=1).astype(np.float16)
        wcat = np.ascontiguousarray(
            wcat.reshape(KT, P, 384).transpose(1, 0, 2))
        qb_ = (ln_b @ w_qkv[:, cs[0]]).astype(np.float32)
        kb_ = (ln_b @ w_qkv[:, cs[1]]).astype(np.float32)
        vb_ = (ln_b @ w_qkv[:, cs[2]]).astype(np.float32)
        in_maps.append({
            "xT": np.ascontiguousarray(x[b].T.astype(np.float16)),
            "wqkv": wcat,
            "wout": np.ascontiguousarray(
                w_out[h0 * D:(h0 + 2) * D]).astype(np.float16),
            "qkb": np.ascontiguousarray(np.stack([qb_, kb_], axis=1)),
            "vb": np.ascontiguousarray(vb_[None, :]),
        })
    return in_maps


def kernel(x, ln_w, ln_b, w_qkv, w_out):
    if "nc" not in _CACHE:
        _CACHE["nc"] = build_nc()
    nc = _CACHE["nc"]
    in_maps = make_in_maps(x, ln_w, ln_b, w_qkv, w_out)
    res = bass_utils.run_bass_kernel_spmd(nc, in_maps,
                                          core_ids=list(range(N_CORES)))
    y = np.zeros((B, N, DIM), np.float32)
    for core in range(N_CORES):
        y[core // 4] += res.results[core]["y"].astype(np.float32)
    return y
